# revision 7
# baseline (speedup 1.0000x reference)
"""Trainium2 Bass kernel for nn_ComplexMixture.

Per batch element b (R = input_real[b] [S,D], I = input_imag[b] [S,D], w [S]):
    out_r = (w*R)^T R + (w*I)^T I        (symmetric)
    out_i = (w*I)^T R - (w*R)^T I        (antisymmetric)

Fold sqrt(w) into both operands (A = sqrt(w)*R, B = sqrt(w)*I) and use the
Gauss 3-multiplication complex product with E = A + B:
    M1 = A^T B,  M2 = B^T A,  M3 = E^T E
    out_r = M3 - M1 - M2
    out_i = M2 - M1
so each output block pair costs 3 PSUM-accumulated matmuls per contraction
chunk instead of 4 (25% less PE time); the combines run on the vector/gpsimd
engines concurrently with the matmul stream.

Sharding: data-parallel over batch, one batch element per NeuronCore (B == 8
== n_cores). Each core runs the identical program on its own slice.

Host marshalling: R/I are cast to fp16 (halves input DMA bytes) and sqrt(w)
is precomputed on host (4K scalars). Matmuls run in fp16 with fp32 PSUM
accumulation; outputs are evacuated as fp16 (halves store DMA bytes) and
upcast on host. Measured L2 relative error vs the fp32 reference ~5e-4.

out_r is symmetric and out_i antisymmetric, so each strictly-lower [384,384]
block is skipped on device (the pass list covers only the upper block
triangle). The host unshard mirrors them with pure transpose copies: out_r's
directly, out_i's from a device-negated side output (oin_out = M1 - M2).

A short burst of dummy matmuls on zeroed tiles runs during the input-DMA head
so the PE HAM clock-gate is already released when real matmuls start. The
first two passes accumulate k=0..2 before either touches k=3, so the
late-arriving k2/k3 chunks don't stall the PE head.
"""

import sys
import types

import numpy as np

# If the environment requests tracing (BASS_TRACE=1) but the image lacks
# antenv.axon_hooks, bass_utils would crash importing it; provide a no-op
# hook registry so tracing degrades gracefully instead.
try:
    import antenv.axon_hooks  # noqa: F401
except ImportError:
    _hooks = types.ModuleType("antenv.axon_hooks")
    _hooks._hook = None
    _hooks.set_axon_ntff_profile_hook = lambda h: setattr(_hooks, "_hook", h)
    _hooks.get_axon_ntff_profile_hook = lambda: _hooks._hook
    sys.modules["antenv.axon_hooks"] = _hooks

import concourse.bacc as bacc
import concourse.bass_utils as bass_utils
import concourse.mybir as mybir
import concourse.tile as tile

B, S, D = 8, 512, 768
P = 128          # SBUF/PSUM partitions; matmul contraction tile
KC = S // P      # 4 contraction chunks per operand
MT = D // P      # 6 output row tiles
NW = 384         # matmul moving free dim (<=512 fp32 PSUM bank)
NB = D // NW     # 2 output column blocks
N_CORES = 8
N_PREWARM = 7    # dummy N=512 matmuls bridging the preamble barrier to the
                 # first real matmuls so the HAM clock gate releases early

# upper-block-triangle passes (m row tile, n 384-col block); strictly-lower
# blocks are mirrored on host from symmetry
PASSES = [(0, 0), (0, 1), (1, 0), (1, 1), (2, 0), (2, 1), (3, 1), (4, 1), (5, 1)]

_CACHE: dict = {}


def _build():
    f32, f16 = mybir.dt.float32, mybir.dt.float16
    BYP = mybir.AluOpType.bypass
    SUB = mybir.AluOpType.subtract
    nc = bacc.Bacc(
        "TRN2", target_bir_lowering=False, debug=False, num_devices=N_CORES
    )
    # Host-packed partition-major: r_in[p, k*D:(k+1)*D] = R[k*P+p, :], so a
    # whole k-chunk group is one DMA with long per-partition descriptors.
    r_d = nc.dram_tensor("r_in", [P, KC * D], f16, kind="ExternalInput").ap()
    i_d = nc.dram_tensor("i_in", [P, KC * D], f16, kind="ExternalInput").ap()
    # sqrt(w) chunks, partition-major (col k = chunk k's 128 scalars)
    s_d = nc.dram_tensor("s_in", [P, KC], f32, kind="ExternalInput").ap()
    or_d = nc.dram_tensor("or_out", [D, D], f16, kind="ExternalOutput").ap()
    oi_d = nc.dram_tensor("oi_out", [D, D], f16, kind="ExternalOutput").ap()
    # negated upper-right block of out_i (= M1 - M2); host transposes it into
    # the skipped lower-left block (out_i is antisymmetric)
    oin_d = nc.dram_tensor("oin_out", [D // 2, NW], f16, kind="ExternalOutput").ap()

    with tile.TileContext(nc) as tc:
        with (
            tc.tile_pool(name="const", bufs=1) as cpool,
            tc.tile_pool(name="stage", bufs=1) as spool,
            tc.tile_pool(name="abc", bufs=1) as apool,
            tc.tile_pool(name="tsb", bufs=2) as tpool,
            tc.tile_pool(name="osb", bufs=2) as opool,
            tc.tile_pool(name="ps", bufs=2, space="PSUM") as pspool,
            tc.tile_pool(name="pw", bufs=1, space="PSUM") as pwpool,
        ):
            # Scale vector first on the gpsimd ring: tiny (2KB), lands before
            # the bulk r23/i23 loads queued behind it.
            s_t = cpool.tile([P, KC], f32, name="s_t")
            nc.gpsimd.dma_start(s_t[:], s_d)

            # PE prewarm: dummy matmuls on zeros bridge the PE from the
            # preamble barrier into the first real matmuls so the HAM
            # activity window sees continuous work and un-throttles early.
            zw = cpool.tile([P, 5 * P], f16, name="zw")
            nc.vector.memset(zw[:], 0.0)
            pw_ps = pwpool.tile([P, 4 * P], f32, name="pw_ps", tag="pw")
            for _ in range(N_PREWARM):
                nc.tensor.matmul(
                    pw_ps[:], zw[:, 0:P], zw[:, P : 5 * P], start=True, stop=True
                )

            # Inputs staggered in consumption order: fine-grained chunks
            # first so k=0/1 land early, the k=2/3 pair as one bigger DMA
            # with longer descriptors. r on the sync HWDGE ring, i on the
            # scalar ring, the late pair on the gpsimd SWDGE ring.
            r0 = spool.tile([P, D], f16, name="r0", tag="r0")
            i0 = spool.tile([P, D], f16, name="i0", tag="i0")
            r1 = spool.tile([P, D], f16, name="r1", tag="r1")
            i1 = spool.tile([P, D], f16, name="i1", tag="i1")
            r23 = spool.tile([P, 2 * D], f16, name="r23", tag="r23")
            i23 = spool.tile([P, 2 * D], f16, name="i23", tag="i23")
            nc.sync.dma_start(r0[:], r_d[:, 0:D])
            nc.scalar.dma_start(i0[:], i_d[:, 0:D])
            nc.sync.dma_start(r1[:], r_d[:, D : 2 * D])
            nc.scalar.dma_start(i1[:], i_d[:, D : 2 * D])
            nc.gpsimd.dma_start(r23[:], r_d[:, 2 * D : 4 * D])
            nc.gpsimd.dma_start(i23[:], i_d[:, 2 * D : 4 * D])

            def rfk(k):
                return (r0[:], r1[:], r23[:, 0:D], r23[:, D : 2 * D])[k]

            def ifk(k):
                return (i0[:], i1[:], i23[:, 0:D], i23[:, D : 2 * D])[k]

            # Per-row scaling on VectorE (fast, tight deadlines); the E sums
            # on the otherwise-idle gpsimd engine.
            At, Bt, Et = [], [], []
            for k in range(KC):
                a = apool.tile([P, D], f16, name=f"A{k}", tag=f"A{k}")
                nc.vector.tensor_scalar_mul(a[:], rfk(k), s_t[:, k : k + 1])
                b = apool.tile([P, D], f16, name=f"B{k}", tag=f"B{k}")
                nc.vector.tensor_scalar_mul(b[:], ifk(k), s_t[:, k : k + 1])
                e = apool.tile([P, D], f16, name=f"E{k}", tag=f"E{k}")
                nc.gpsimd.tensor_add(e[:], a[:], b[:])
                At.append(a)
                Bt.append(b)
                Et.append(e)

            def nsl(n):
                return slice(n * NW, (n + 1) * NW)

            ps_of = {}

            def mm_group(p, k):
                """Emit the 3 matmuls of pass p for contraction chunk k."""
                m, n = PASSES[p]
                ms = slice(m * P, (m + 1) * P)
                M1, M2, M3 = ps_of[p]
                st, sp = (k == 0), (k == KC - 1)
                # stop group ordered M3 first: the first evac op needs M3/M1
                order = (
                    [(M3, Et[k], Et[k]), (M1, At[k], Bt[k]), (M2, Bt[k], At[k])]
                    if sp
                    else [(M1, At[k], Bt[k]), (M2, Bt[k], At[k]), (M3, Et[k], Et[k])]
                )
                for dst, lt, rt in order:
                    nc.tensor.matmul(
                        dst[:], lt[:, ms], rt[:, nsl(n)], start=st, stop=sp
                    )

            def evac(p):
                """Combine pass p's PSUM banks and store (fp16). HW allows
                only one PSUM operand per vector op, so M1 is first copied
                to SBUF (u) on the otherwise-idle scalar engine."""
                m, n = PASSES[p]
                ms = slice(m * P, (m + 1) * P)
                M1, M2, M3 = ps_of[p]
                u = tpool.tile([P, NW], f32, name=f"u{p}", tag="u")
                t = tpool.tile([P, NW], f32, name=f"t{p}", tag="t")
                or_sb = opool.tile([P, NW], f16, name=f"or{p}", tag="or_sb")
                oi_sb = opool.tile([P, NW], f16, name=f"oi{p}", tag="oi_sb")
                nc.scalar.copy(u[:], M1[:])
                nc.vector.scalar_tensor_tensor(t[:], M3[:], 0.0, u[:], BYP, SUB)
                nc.vector.scalar_tensor_tensor(or_sb[:], t[:], 0.0, M2[:], BYP, SUB)
                nc.sync.dma_start(or_d[ms, nsl(n)], or_sb[:])
                nc.vector.scalar_tensor_tensor(oi_sb[:], M2[:], 0.0, u[:], BYP, SUB)
                nc.scalar.dma_start(oi_d[ms, nsl(n)], oi_sb[:])
                if n == 1 and m < MT // 2:
                    # negated out_i block for the host-side antisymmetric
                    # mirror; gpsimd can't read PSUM, so negate the fp16
                    # SBUF copy (exact sign flip)
                    oin_sb = opool.tile([P, NW], f16, name=f"oin{p}", tag="oin_sb")
                    nc.gpsimd.tensor_sub(oin_sb[:], zw[:, 0:NW], oi_sb[:])
                    nc.gpsimd.dma_start(oin_d[ms, :], oin_sb[:])

            # Passes 0/1 accumulate k=0..2 before either touches k=3, so the
            # late-arriving r23/i23 chunks don't stall the PE head. From
            # pass 2 on, inputs are resident and passes run straight through.
            for p in (0, 1):
                ps_of[p] = tuple(
                    pspool.tile([P, NW], f32, name=f"M{t_}_{p}", tag=f"M{t_}")
                    for t_ in (1, 2, 3)
                )
            for k in range(KC - 1):
                mm_group(0, k)
            for k in range(KC - 1):
                mm_group(1, k)
            mm_group(0, KC - 1)
            evac(0)
            mm_group(1, KC - 1)
            evac(1)
            for p in range(2, len(PASSES)):
                ps_of[p] = tuple(
                    pspool.tile([P, NW], f32, name=f"M{t_}_{p}", tag=f"M{t_}")
                    for t_ in (1, 2, 3)
                )
                for k in range(KC):
                    mm_group(p, k)
                evac(p)

    nc.compile()
    return nc


def get_nc():
    if "nc" not in _CACHE:
        _CACHE["nc"] = _build()
    return _CACHE["nc"]


def make_in_maps(input_real, input_imag, weight):
    input_real = np.asarray(input_real)
    input_imag = np.asarray(input_imag)
    weight = np.asarray(weight, dtype=np.float32)
    # pack [S, D] -> [P, KC*D]: row p holds chunks k=0..KC-1 concatenated
    r16 = (
        input_real.astype(np.float16)
        .reshape(B, KC, P, D)
        .transpose(0, 2, 1, 3)
        .reshape(B, P, KC * D)
    )
    i16 = (
        input_imag.astype(np.float16)
        .reshape(B, KC, P, D)
        .transpose(0, 2, 1, 3)
        .reshape(B, P, KC * D)
    )
    # [B, P, KC]: col k = sqrt(w) for chunk k
    s_pack = np.sqrt(weight).astype(np.float32).reshape(B, KC, P).transpose(0, 2, 1)
    return [
        {
            "r_in": np.ascontiguousarray(r16[b]),
            "i_in": np.ascontiguousarray(i16[b]),
            "s_in": np.ascontiguousarray(s_pack[b]),
        }
        for b in range(B)
    ]


def unshard_single(or_np, oi_np, oin_np):
    """fp16 device outputs -> full fp32 [D,D] pair, mirroring the skipped
    strictly-lower blocks (pure transpose copies of device-computed data)."""
    out_r = np.asarray(or_np).astype(np.float32)
    out_i = np.asarray(oi_np).astype(np.float32)
    out_r[NW:D, 0:NW] = out_r[0:NW, NW:D].T
    out_i[NW:D, 0:NW] = np.asarray(oin_np).astype(np.float32).T
    return out_r, out_i


def run(input_real, input_imag, weight, **spmd_kwargs):
    nc = get_nc()
    res = bass_utils.run_bass_kernel_spmd(
        nc,
        make_in_maps(input_real, input_imag, weight),
        core_ids=list(range(N_CORES)),
        **spmd_kwargs,
    )
    outs = [
        unshard_single(
            res.results[b]["or_out"], res.results[b]["oi_out"],
            res.results[b]["oin_out"],
        )
        for b in range(B)
    ]
    out_r = np.stack([o[0] for o in outs])
    out_i = np.stack([o[1] for o in outs])
    return (out_r, out_i), res


def kernel(input_real, input_imag, weight):
    (out_r, out_i), _ = run(input_real, input_imag, weight)
    return (out_r, out_i)
